# revision 21
# baseline (speedup 1.0000x reference)
"""2-layer GCN (GCNConv x2, relu between) on 8 Trainium2 NeuronCores.

Strategy (graph-parallel, per the sharding hint):
  - Nodes are partitioned into 8 contiguous shards by dst; each core owns the
    edges incident (by dst) to its shard, plus one self-loop token per node.
  - Aggregation is transform-last:  out = dinv_d * ((sum_e dinv_src*x[src]) @ W) + b
    so the per-edge work is a pure row gather (dma_gather) + a segment-sum
    done on the TensorEngine via narrow one-hot matrices built on the DVE.
  - Self-loops are ordinary edges (src == dst).
  - Layer 1 gathers raw x rows and folds dinv[src] into the one-hot values.
    Layer 2 gathers h1p = dinv * relu(layer1) rows (pre-scaled), one-hot is 0/1.
  - h1p shards are AllGather'd across the 8 cores between the layers.

The Bass program is SPMD: one program, per-core data. All per-core arrays are
padded to common shapes; token slots are padded with (idx=0, dstf=-1, dinvs=0)
so pads gather real rows but contribute exactly zero.
"""

import sys

if "/opt/trn_rl_repo" not in sys.path:
    sys.path.insert(0, "/opt/trn_rl_repo")

import numpy as np

P = 128


def _cdiv(a, b):
    return (a + b - 1) // b


def preprocess(x, edge_index, n_cores, split):
    """Host-side graph preprocessing. Returns (meta, per_core_arrays)."""
    N, IN = x.shape
    src = edge_index[0].astype(np.int64)
    dst = edge_index[1].astype(np.int64)
    SH = N // n_cores
    assert SH * n_cores == N
    W = _cdiv(SH, P)

    deg = (np.bincount(dst, minlength=N).astype(np.float32) + 1.0).astype(np.float32)
    dinv = (1.0 / np.sqrt(deg)).astype(np.float32)

    # per (core, window, grp) token lists, sorted by local dst
    # grp 0: src < split (gather source x[:split]); grp 1: src >= split
    tok = {}  # (c, w, g) -> (src_arr, dloc_arr)
    cnt = np.zeros((n_cores, W, 2), np.int64)
    for c in range(n_cores):
        m = (dst >= c * SH) & (dst < (c + 1) * SH)
        s_c = np.concatenate([src[m], np.arange(c * SH, (c + 1) * SH)])
        d_c = np.concatenate([dst[m] - c * SH, np.arange(SH)])
        w_c = d_c >> 7
        dl_c = d_c & 127
        g_c = (s_c >= split).astype(np.int64)
        order = np.lexsort((dl_c, g_c, w_c))
        s_c, dl_c, w_c, g_c = s_c[order], dl_c[order], w_c[order], g_c[order]
        # boundaries
        key = (w_c * 2 + g_c)
        for kv in range(2 * W):
            sel = key == kv
            w, g = kv // 2, kv % 2
            tok[(c, w, g)] = (s_c[sel], dl_c[sel])
            cnt[c, w, g] = sel.sum()

    LO_T = [int(max(_cdiv(int(cnt[c, w, 0]), P) for c in range(n_cores))) for w in range(W)]
    HI_T = [int(max(_cdiv(int(cnt[c, w, 1]), P) for c in range(n_cores))) for w in range(W)]

    # chunking: CHUNK_W windows per chunk; stream order per chunk:
    #   [w0.lo tiles, w1.lo, ...][w0.hi, w1.hi, ...]
    CHUNK_W = 4
    chunks = []  # list of dicts describing each chunk
    NT = 0  # total tiles
    tile_meta = []  # per global tile: dict(w, first, last)
    for c0 in range(0, W, CHUNK_W):
        ws = list(range(c0, min(c0 + CHUNK_W, W)))
        lo_tiles = sum(LO_T[w] for w in ws)
        hi_tiles = sum(HI_T[w] for w in ws)
        ch = {
            "ws": ws,
            "t0": NT,
            "lo_tiles": lo_tiles,
            "hi_tiles": hi_tiles,
            "tiles": lo_tiles + hi_tiles,
        }
        # per-tile window assignment
        tmeta = []
        for g in (0, 1):
            for w in ws:
                ntl = (LO_T, HI_T)[g][w]
                for i in range(ntl):
                    tmeta.append({"w": w, "g": g, "i": i})
        # mark first/last tile of each window (within this chunk; windows don't
        # span chunks)
        seen_first = set()
        last_idx = {}
        for ti, tm in enumerate(tmeta):
            w = tm["w"]
            tm["first"] = w not in seen_first
            seen_first.add(w)
            last_idx[w] = ti
        for ti, tm in enumerate(tmeta):
            tm["last"] = last_idx[tm["w"]] == ti
        ch["tmeta"] = tmeta
        # per-window iteration order (lo tiles then hi tiles of that window)
        ch["wtiles"] = {
            w: [ti for ti, tm in enumerate(tmeta) if tm["w"] == w] for w in ws
        }
        chunks.append(ch)
        tile_meta.extend(tmeta)
        NT += len(tmeta)

    NTOK = NT * P

    # build per-core flat slot arrays + per-tile span (union over cores)
    d0_arr = np.full(NT, 127, np.int64)
    d1_arr = np.zeros(NT, np.int64)
    per_core = []
    for c in range(n_cores):
        idx_local = np.zeros(NTOK, np.int32)
        dstf = np.full(NTOK, -1.0, np.float32)
        dinvs = np.zeros(NTOK, np.float32)
        for ch in chunks:
            base = ch["t0"]
            for ti, tm in enumerate(ch["tmeta"]):
                gt = base + ti
                w, g, i = tm["w"], tm["g"], tm["i"]
                s_all, dl_all = tok[(c, w, g)]
                a, b = i * P, min((i + 1) * P, len(s_all))
                n = max(0, b - a)
                o = gt * P
                if n > 0:
                    sv = s_all[a:b]
                    dv = dl_all[a:b]
                    idx_local[o : o + n] = sv - (split if g else 0)
                    dstf[o : o + n] = dv.astype(np.float32)
                    dinvs[o : o + n] = dinv[sv]
                    d0_arr[gt] = min(d0_arr[gt], int(dv.min()))
                    d1_arr[gt] = max(d1_arr[gt], int(dv.max()))
                if n < P:
                    # pads: idx 0 is valid in both lo and hi views
                    idx_local[o + n : o + P] = 0
        # int16 wrap layout [128, NTOK//16], token j -> (j%16, j//16), replicated x8
        assert idx_local.max() < 32768 and idx_local.min() >= 0
        i16 = idx_local.astype(np.int16).reshape(NTOK // 16, 16).T  # [16, NTOK//16]
        i16 = np.tile(i16, (8, 1)).copy()  # [128, NTOK//16]
        dstf_sb = dstf.reshape(NT, P).T.copy()  # [128, NT]
        dinvs_sb = dinvs.reshape(NT, P).T.copy()
        # per-window per-node dinv [128, W]
        dinvw = np.ones((P, W), np.float32)
        for w in range(W):
            dw = min(P, SH - w * P)
            dinvw[:dw, w] = dinv[c * SH + w * P : c * SH + w * P + dw]
        per_core.append(
            {"src16": i16, "dstf": dstf_sb, "dinvs": dinvs_sb, "dinvw": dinvw}
        )

    # tile spans (compile-time, shared across cores)
    spans = []
    for gt in range(NT):
        if tile_meta[gt]["first"]:
            spans.append((0, P))
        else:
            d0 = int(d0_arr[gt]) & ~1
            d1 = int(d1_arr[gt])
            if d1 < d0:  # all-pad tile on every core
                d0, d1 = 0, 1
            end = min(P, (d1 + 2) & ~1)
            spans.append((d0, end - d0))
    for gt in range(NT):
        tile_meta[gt]["d0"], tile_meta[gt]["span"] = spans[gt]

    meta = {
        "N": N,
        "IN": IN,
        "SH": SH,
        "W": W,
        "NT": NT,
        "split": split,
        "n_cores": n_cores,
        "chunks": chunks,
        "LO_T": LO_T,
        "HI_T": HI_T,
    }
    return meta, per_core


def build_kernel(tc, outs, ins, meta, HID, OUT):
    """Trace the SPMD program. ins order:
    x, w1, w2, b1bc, b2bc, iota, dinvw, src16, dstf, dinvs"""
    import concourse.bass as bass
    from concourse import mybir

    nc = tc.nc
    x_ap, w1_ap, w2_ap, b1_ap, b2_ap, iota_ap, dinvw_ap, src16_ap, dstf_ap, dinvs_ap = ins
    out_ap = outs[0]

    N, IN, SH, W, NT, SPLIT = (
        meta["N"],
        meta["IN"],
        meta["SH"],
        meta["W"],
        meta["NT"],
        meta["split"],
    )
    n_cores = meta["n_cores"]
    chunks = meta["chunks"]
    f32 = mybir.dt.float32
    AT = mybir.ActivationFunctionType
    OP = mybir.AluOpType

    MAX_CT = max(ch["tiles"] for ch in chunks)

    import contextlib

    with contextlib.ExitStack() as ctx:
        const = ctx.enter_context(tc.tile_pool(name="const", bufs=1))
        dram = ctx.enter_context(tc.tile_pool(name="dram", bufs=1, space="DRAM"))
        ohp = ctx.enter_context(tc.tile_pool(name="oh", bufs=12))
        psum_s = ctx.enter_context(tc.tile_pool(name="psum_s", bufs=3, space="PSUM"))
        psum_g = ctx.enter_context(tc.tile_pool(name="psum_g", bufs=2, space="PSUM"))
        tailp = ctx.enter_context(tc.tile_pool(name="tail", bufs=6))

        # persistent constants in SBUF
        iota_sb = const.tile([P, P], f32)
        nc.sync.dma_start(iota_sb[:], iota_ap[:])
        w1_sb = const.tile([IN, HID], f32)
        nc.sync.dma_start(w1_sb[:], w1_ap[:])
        w2_sb = const.tile([HID, OUT], f32)
        nc.sync.dma_start(w2_sb[:], w2_ap[:])
        b1_sb = const.tile([P, HID], f32)
        nc.sync.dma_start(b1_sb[:], b1_ap[:])
        b2_sb = const.tile([P, OUT], f32)
        nc.sync.dma_start(b2_sb[:], b2_ap[:])
        dinvw_sb = const.tile([P, W], f32)
        nc.sync.dma_start(dinvw_sb[:], dinvw_ap[:])
        src16_sb = const.tile([P, NT * 8], mybir.dt.int16)
        nc.sync.dma_start(src16_sb[:], src16_ap[:])
        dstf_sb = const.tile([P, NT], f32)
        nc.sync.dma_start(dstf_sb[:], dstf_ap[:])
        dinvs_sb = const.tile([P, NT], f32)
        nc.sync.dma_start(dinvs_sb[:], dinvs_ap[:])

        h1p_shard = dram.tile([SH, HID], f32)
        h1p_full = dram.tile([N, HID], f32)

        import os as _os
        _VARIANT = _os.environ.get("GCN_VARIANT", "full")
        NSWQ = int(_os.environ.get("GCN_NSWQ", "4"))
        TOKBUFS = int(_os.environ.get("GCN_TOKBUFS", "4"))
        qctr = [0]
        tokp_shared = ctx.enter_context(tc.tile_pool(name="tokp", bufs=TOKBUFS))

        def layer(F, src_dram, is_l1):
            """One GCN layer: gathers F-wide rows, segment-sums, returns via
            writer callback per window."""
            tokp = tokp_shared
            for ch in chunks:
                ct = ch["tiles"]
                t0 = ch["t0"]
                tokt = tokp.tile(
                    [P, MAX_CT, F], f32, tag="tok",
                    name=f"tok_{1 if is_l1 else 2}_{ch['t0']}",
                )
                # gather calls: lo then hi sections of this chunk
                nlo, nhi = ch["lo_tiles"], ch["hi_tiles"]
                GMAX = 8  # <=1024 idxs per call keeps single_packet mode legal
                for a0, n_all, src_ap in (
                    (0, nlo, src_dram[0:SPLIT, :]),
                    (nlo, nhi, src_dram[SPLIT:, :]),
                ):
                    for a in range(a0, a0 + n_all, GMAX):
                        b = min(a + GMAX, a0 + n_all)
                        nc.gpsimd.dma_gather(
                            out_ap=tokt[:, a:b, :],
                            in_ap=src_ap,
                            idxs_ap=src16_sb[:, (t0 + a) * 8 : (t0 + b) * 8],
                            num_idxs=(b - a) * P,
                            num_idxs_reg=(b - a) * P,
                            elem_size=F,
                            single_packet=True,
                            queue_num=qctr[0] % NSWQ,
                        )
                        qctr[0] += 1
                # per-window segment-sum matmuls
                win_psum = {}
                for w in ch["ws"]:
                  for ti in ch["wtiles"][w]:
                    tm = ch["tmeta"][ti]
                    gt = t0 + ti
                    d0, span = tm["d0"], tm["span"]
                    if tm["first"]:
                        win_psum[w] = psum_s.tile(
                            [P if is_l1 else HID, P], f32, tag="S", name=f"S_{w}"
                        )
                    oh = ohp.tile([P, P], f32, tag="oh")
                    if is_l1:
                        nc.vector.tensor_scalar(
                            out=oh[:, :span],
                            in0=iota_sb[:, d0 : d0 + span],
                            scalar1=dstf_sb[:, gt : gt + 1],
                            scalar2=dinvs_sb[:, gt : gt + 1],
                            op0=OP.is_equal,
                            op1=OP.mult,
                        )
                    else:
                        nc.vector.tensor_scalar(
                            out=oh[:, :span],
                            in0=iota_sb[:, d0 : d0 + span],
                            scalar1=dstf_sb[:, gt : gt + 1],
                            scalar2=None,
                            op0=OP.is_equal,
                        )
                    nc.tensor.matmul(
                        out=win_psum[w][:, d0 : d0 + span],
                        lhsT=tokt[:, ti, :],
                        rhs=oh[:, :span],
                        start=tm["first"],
                        stop=tm["last"],
                        skip_group_check=True,
                    )
                    if tm["last"]:
                        tail(w, win_psum[w], F, is_l1)
                    if _VARIANT == "gatheronly":
                        break

        def tail(w, s_psum, F, is_l1):
            dw = min(P, SH - w * P)
            # S.T -> SBUF
            p1t = tailp.tile([F, P], f32, tag=f"pt{1 if is_l1 else 2}")
            nc.vector.tensor_copy(out=p1t[:], in_=s_psum[:])
            wsb = w1_sb if is_l1 else w2_sb
            HO = HID if is_l1 else OUT
            g = psum_g.tile([P, HO], f32, tag="G")
            nc.tensor.matmul(
                out=g[:dw, :],
                lhsT=p1t[:, :dw],
                rhs=wsb[:],
                start=True,
                stop=True,
            )
            t1 = tailp.tile([P, HO], f32, tag=f"t1_{1 if is_l1 else 2}")
            # t1 = dinv_d * (S @ W)
            nc.vector.tensor_scalar(
                out=t1[:dw, :],
                in0=g[:dw, :],
                scalar1=dinvw_sb[:dw, w : w + 1],
                scalar2=None,
                op0=OP.mult,
            )
            t2 = tailp.tile([P, HO], f32, tag=f"t2_{1 if is_l1 else 2}")
            bsb = b1_sb if is_l1 else b2_sb
            nc.vector.tensor_tensor(
                out=t2[:dw, :], in0=t1[:dw, :], in1=bsb[:dw, :], op=OP.add
            )
            if is_l1:
                # h1p = dinv * relu(t2) == relu(dinv * t2)
                h1p_t = tailp.tile([P, HID], f32, tag="h1p")
                nc.scalar.activation(
                    out=h1p_t[:dw, :],
                    in_=t2[:dw, :],
                    func=AT.Relu,
                    scale=dinvw_sb[:dw, w : w + 1],
                )
                nc.sync.dma_start(
                    h1p_shard[w * P : w * P + dw, :], h1p_t[:dw, :]
                )
            else:
                nc.sync.dma_start(out_ap[w * P : w * P + dw, :], t2[:dw, :])

        layer(IN, x_ap, True)
        if _VARIANT in ("l1only", "gatheronly"):
            return
        if _os.environ.get("GCN_NOCOLL", "0") == "1":
            for _c in range(n_cores):
                nc.gpsimd.dma_start(
                    h1p_full[_c * SH : (_c + 1) * SH, :], h1p_shard[:]
                )
        else:
            nc.gpsimd.collective_compute(
                "AllGather",
                mybir.AluOpType.bypass,
                replica_groups=[list(range(n_cores))],
                ins=[h1p_shard[:]],
                outs=[h1p_full[:]],
            )
        layer(HID, h1p_full[:], False)


def make_inputs(x, W1, b1, W2, b2, meta, per_core):
    """Build the per-core input pytrees (ordered list)."""
    IN, W, NT = meta["IN"], meta["W"], meta["NT"]
    HID = W1.shape[1]
    OUT = W2.shape[1]
    iota = np.tile(np.arange(P, dtype=np.float32)[None, :], (P, 1))
    b1bc = np.tile(np.asarray(b1, np.float32)[None, :], (P, 1))
    b2bc = np.tile(np.asarray(b2, np.float32)[None, :], (P, 1))
    ins_list = []
    for pc in per_core:
        ins_list.append(
            [
                np.asarray(x, np.float32),
                np.asarray(W1, np.float32),
                np.asarray(W2, np.float32),
                b1bc,
                b2bc,
                iota,
                pc["dinvw"],
                pc["src16"],
                pc["dstf"],
                pc["dinvs"],
            ]
        )
    return ins_list


IN_NAMES = [
    "x", "w1", "w2", "b1bc", "b2bc", "iota", "dinvw", "src16", "dstf", "dinvs",
]


def run(x, W1, b1, W2, b2, edge_index, n_cores=8, split=32768, trace=False):
    import concourse.tile as tile
    from concourse import bacc, bass_utils, mybir
    from concourse.bass_interp import get_hw_module

    meta, per_core = preprocess(np.asarray(x), np.asarray(edge_index), n_cores, split)
    HID = W1.shape[1]
    OUT = W2.shape[1]
    SH = meta["SH"]
    ins_list = make_inputs(x, W1, b1, W2, b2, meta, per_core)

    import os as _os
    nc = bacc.Bacc(
        "TRN2", target_bir_lowering=False, debug=False, num_devices=n_cores,
        num_swdge_queues=int(_os.environ.get("GCN_NSWQ", "4")),
    )
    in_aps = [
        nc.dram_tensor(nm, list(a.shape), mybir.dt.from_np(a.dtype),
                       kind="ExternalInput").ap()
        for nm, a in zip(IN_NAMES, ins_list[0])
    ]
    out_t = nc.dram_tensor("out", [SH, OUT], mybir.dt.float32,
                           kind="ExternalOutput")
    with tile.TileContext(nc) as tc:
        build_kernel(tc, [out_t.ap()], in_aps, meta, HID, OUT)
    nc.compile()

    in_maps = [
        {nm: np.ascontiguousarray(a) for nm, a in zip(IN_NAMES, arrs)}
        for arrs in ins_list
    ]
    old_m = nc.m
    nc.m = get_hw_module(nc.m)
    try:
        res = bass_utils.run_bass_kernel_spmd(
            nc, in_maps, core_ids=list(range(n_cores)), trace=False
        )
        bench_ns = _bench(nc, in_maps, n_cores) if trace else None
    finally:
        nc.m = old_m
    out = np.concatenate([res.results[c]["out"] for c in range(n_cores)], axis=0)
    return out, res, bench_ns


def _bench(nc, in_maps, n_cores, iters=30):
    """Repeat-execute the compiled NEFF with device-resident inputs and
    return the min wall-clock ns per execution (upper bound on HW time)."""
    import time

    import jax
    from concourse import bass2jax, mybir
    from jax.sharding import Mesh, PartitionSpec
    from jax.experimental.shard_map import shard_map

    part_name = nc.partition_id_tensor.name if nc.partition_id_tensor else None
    in_names, out_names, out_avals, zero_outs = [], [], [], []
    for alloc in nc.m.functions[0].allocations:
        if not isinstance(alloc, bass2jax.mybir.MemoryLocationSet):
            continue
        name = alloc.memorylocations[0].name
        if alloc.kind == "ExternalInput":
            if name != part_name:
                in_names.append(name)
        elif alloc.kind == "ExternalOutput":
            out_names.append(name)
            shape = tuple(alloc.tensor_shape)
            dtype = bass2jax.mybir.dt.np(alloc.dtype)
            out_avals.append(jax.core.ShapedArray(shape, dtype))
            zero_outs.append(np.zeros(shape, dtype))
    n_params = len(in_names)
    all_names = in_names + out_names
    if part_name is not None:
        all_names = all_names + [part_name]

    def _make_body(nchain):
        def _body(*args):
            ins = list(args[:n_params])
            outs = list(args[n_params:])
            for _ in range(nchain):
                operands = ins + outs
                if part_name is not None:
                    operands.append(bass2jax.partition_id_tensor())
                outs = list(
                    bass2jax._bass_exec_p.bind(
                        *operands,
                        out_avals=tuple(out_avals),
                        in_names=tuple(all_names),
                        out_names=tuple(out_names),
                        lowering_input_output_aliases=(),
                        sim_require_finite=True,
                        sim_require_nnan=True,
                        nc=nc,
                    )
                )
            return tuple(outs)

        return _body

    devices = jax.devices()[:n_cores]
    mesh = Mesh(np.asarray(devices), ("core",))
    nio = n_params + len(out_names)
    sh = jax.sharding.NamedSharding(mesh, PartitionSpec("core"))
    concat_in = [
        jax.device_put(
            np.concatenate([in_maps[c][nm] for c in range(n_cores)], axis=0), sh
        )
        for nm in in_names
    ]
    concat_zero = [
        jax.device_put(np.zeros((n_cores * z.shape[0], *z.shape[1:]), z.dtype), sh)
        for z in zero_outs
    ]

    def time_chain(nchain, reps):
        fn = jax.jit(
            shard_map(
                _make_body(nchain),
                mesh=mesh,
                in_specs=(PartitionSpec("core"),) * nio,
                out_specs=(PartitionSpec("core"),) * len(out_names),
                check_rep=False,
            ),
            keep_unused=True,
        )
        r = fn(*concat_in, *concat_zero)
        jax.block_until_ready(r)
        best = float("inf")
        for _ in range(reps):
            t0 = time.perf_counter()
            r = fn(*concat_in, *concat_zero)
            jax.block_until_ready(r)
            best = min(best, time.perf_counter() - t0)
        return best

    t1 = time_chain(1, iters)
    # dispatch-overhead baseline: trivial jitted op over the same resident input
    base_fn = jax.jit(lambda a: a[0:1, 0:1] * 2.0)
    r = base_fn(concat_in[0])
    jax.block_until_ready(r)
    tb = float("inf")
    for _ in range(iters):
        t0 = time.perf_counter()
        r = base_fn(concat_in[0])
        jax.block_until_ready(r)
        tb = min(tb, time.perf_counter() - t0)
    per_exec = max(0.0, t1 - tb)
    print(f"[bench] kernel={t1*1e3:.3f}ms baseline={tb*1e3:.3f}ms "
          f"delta={per_exec*1e6:.1f}us")
    return int(per_exec * 1e9)


def kernel(x, W1, b1, W2, b2, edge_index):
    out, _, _ = run(
        np.asarray(x, np.float32),
        np.asarray(W1, np.float32),
        np.asarray(b1, np.float32),
        np.asarray(W2, np.float32),
        np.asarray(b2, np.float32),
        np.asarray(edge_index, np.int32),
    )
    return out


# revision 22
# speedup vs baseline: 1.1334x; 1.1334x over previous
"""2-layer GCN (GCNConv x2, relu between) on 8 Trainium2 NeuronCores.

Strategy (graph-parallel, per the sharding hint):
  - Nodes are partitioned into 8 contiguous shards by dst; each core owns the
    edges incident (by dst) to its shard, plus one self-loop token per node.
  - Aggregation is transform-last:  out = dinv_d * ((sum_e dinv_src*x[src]) @ W) + b
    so the per-edge work is a pure row gather (dma_gather) + a segment-sum
    done on the TensorEngine via narrow one-hot matrices built on the DVE.
  - Self-loops are ordinary edges (src == dst).
  - Layer 1 gathers raw x rows and folds dinv[src] into the one-hot values.
    Layer 2 gathers h1p = dinv * relu(layer1) rows (pre-scaled), one-hot is 0/1.
  - h1p shards are AllGather'd across the 8 cores between the layers.

The Bass program is SPMD: one program, per-core data. All per-core arrays are
padded to common shapes; token slots are padded with (idx=0, dstf=-1, dinvs=0)
so pads gather real rows but contribute exactly zero.
"""

import sys

if "/opt/trn_rl_repo" not in sys.path:
    sys.path.insert(0, "/opt/trn_rl_repo")

import numpy as np

P = 128


def _cdiv(a, b):
    return (a + b - 1) // b


def preprocess(x, edge_index, n_cores, split):
    """Host-side graph preprocessing. Returns (meta, per_core_arrays)."""
    N, IN = x.shape
    src = edge_index[0].astype(np.int64)
    dst = edge_index[1].astype(np.int64)
    SH = N // n_cores
    assert SH * n_cores == N
    W = _cdiv(SH, P)

    deg = (np.bincount(dst, minlength=N).astype(np.float32) + 1.0).astype(np.float32)
    dinv = (1.0 / np.sqrt(deg)).astype(np.float32)

    # per (core, window, grp) token lists, sorted by local dst
    # grp 0: src < split (gather source x[:split]); grp 1: src >= split
    tok = {}  # (c, w, g) -> (src_arr, dloc_arr)
    cnt = np.zeros((n_cores, W, 2), np.int64)
    for c in range(n_cores):
        m = (dst >= c * SH) & (dst < (c + 1) * SH)
        s_c = np.concatenate([src[m], np.arange(c * SH, (c + 1) * SH)])
        d_c = np.concatenate([dst[m] - c * SH, np.arange(SH)])
        w_c = d_c >> 7
        dl_c = d_c & 127
        g_c = (s_c >= split).astype(np.int64)
        order = np.lexsort((dl_c, g_c, w_c))
        s_c, dl_c, w_c, g_c = s_c[order], dl_c[order], w_c[order], g_c[order]
        # boundaries
        key = (w_c * 2 + g_c)
        for kv in range(2 * W):
            sel = key == kv
            w, g = kv // 2, kv % 2
            tok[(c, w, g)] = (s_c[sel], dl_c[sel])
            cnt[c, w, g] = sel.sum()

    LO_T = [int(max(_cdiv(int(cnt[c, w, 0]), P) for c in range(n_cores))) for w in range(W)]
    HI_T = [int(max(_cdiv(int(cnt[c, w, 1]), P) for c in range(n_cores))) for w in range(W)]

    # chunking: CHUNK_W windows per chunk; stream order per chunk:
    #   [w0.lo tiles, w1.lo, ...][w0.hi, w1.hi, ...]
    CHUNK_W = 4
    chunks = []  # list of dicts describing each chunk
    NT = 0  # total tiles
    tile_meta = []  # per global tile: dict(w, first, last)
    for c0 in range(0, W, CHUNK_W):
        ws = list(range(c0, min(c0 + CHUNK_W, W)))
        lo_tiles = sum(LO_T[w] for w in ws)
        hi_tiles = sum(HI_T[w] for w in ws)
        ch = {
            "ws": ws,
            "t0": NT,
            "lo_tiles": lo_tiles,
            "hi_tiles": hi_tiles,
            "tiles": lo_tiles + hi_tiles,
        }
        # per-tile window assignment
        tmeta = []
        for g in (0, 1):
            for w in ws:
                ntl = (LO_T, HI_T)[g][w]
                for i in range(ntl):
                    tmeta.append({"w": w, "g": g, "i": i})
        # mark first/last tile of each window (within this chunk; windows don't
        # span chunks)
        seen_first = set()
        last_idx = {}
        for ti, tm in enumerate(tmeta):
            w = tm["w"]
            tm["first"] = w not in seen_first
            seen_first.add(w)
            last_idx[w] = ti
        for ti, tm in enumerate(tmeta):
            tm["last"] = last_idx[tm["w"]] == ti
        ch["tmeta"] = tmeta
        # per-window iteration order (lo tiles then hi tiles of that window)
        ch["wtiles"] = {
            w: [ti for ti, tm in enumerate(tmeta) if tm["w"] == w] for w in ws
        }
        chunks.append(ch)
        tile_meta.extend(tmeta)
        NT += len(tmeta)

    NTOK = NT * P

    # build per-core flat slot arrays + per-tile span (union over cores)
    d0_arr = np.full(NT, 127, np.int64)
    d1_arr = np.zeros(NT, np.int64)
    per_core = []
    for c in range(n_cores):
        idx_local = np.zeros(NTOK, np.int32)
        dstf = np.full(NTOK, -1.0, np.float32)
        dinvs = np.zeros(NTOK, np.float32)
        for ch in chunks:
            base = ch["t0"]
            for ti, tm in enumerate(ch["tmeta"]):
                gt = base + ti
                w, g, i = tm["w"], tm["g"], tm["i"]
                s_all, dl_all = tok[(c, w, g)]
                a, b = i * P, min((i + 1) * P, len(s_all))
                n = max(0, b - a)
                o = gt * P
                if n > 0:
                    sv = s_all[a:b]
                    dv = dl_all[a:b]
                    idx_local[o : o + n] = sv - (split if g else 0)
                    dstf[o : o + n] = dv.astype(np.float32)
                    dinvs[o : o + n] = dinv[sv]
                    d0_arr[gt] = min(d0_arr[gt], int(dv.min()))
                    d1_arr[gt] = max(d1_arr[gt], int(dv.max()))
                if n < P:
                    # pads: idx 0 is valid in both lo and hi views
                    idx_local[o + n : o + P] = 0
        # int16 wrap layout [128, NTOK//16], token j -> (j%16, j//16), replicated x8
        assert idx_local.max() < 32768 and idx_local.min() >= 0
        i16 = idx_local.astype(np.int16).reshape(NTOK // 16, 16).T  # [16, NTOK//16]
        i16 = np.tile(i16, (8, 1)).copy()  # [128, NTOK//16]
        dstf_sb = dstf.reshape(NT, P).T.copy()  # [128, NT]
        dinvs_sb = dinvs.reshape(NT, P).T.copy()
        # per-window per-node dinv [128, W]
        dinvw = np.ones((P, W), np.float32)
        for w in range(W):
            dw = min(P, SH - w * P)
            dinvw[:dw, w] = dinv[c * SH + w * P : c * SH + w * P + dw]
        per_core.append(
            {"src16": i16, "dstf": dstf_sb, "dinvs": dinvs_sb, "dinvw": dinvw}
        )

    # tile spans (compile-time, shared across cores)
    spans = []
    for gt in range(NT):
        if tile_meta[gt]["first"]:
            spans.append((0, P))
        else:
            d0 = int(d0_arr[gt]) & ~1
            d1 = int(d1_arr[gt])
            if d1 < d0:  # all-pad tile on every core
                d0, d1 = 0, 1
            end = min(P, (d1 + 2) & ~1)
            spans.append((d0, end - d0))
    for gt in range(NT):
        tile_meta[gt]["d0"], tile_meta[gt]["span"] = spans[gt]

    meta = {
        "N": N,
        "IN": IN,
        "SH": SH,
        "W": W,
        "NT": NT,
        "split": split,
        "n_cores": n_cores,
        "chunks": chunks,
        "LO_T": LO_T,
        "HI_T": HI_T,
    }
    return meta, per_core


def build_kernel(tc, outs, ins, meta, HID, OUT):
    """Trace the SPMD program. ins order:
    x, w1, w2, b1bc, b2bc, iota, dinvw, src16, dstf, dinvs"""
    import concourse.bass as bass
    from concourse import mybir

    nc = tc.nc
    x_ap, w1_ap, w2_ap, b1_ap, b2_ap, iota_ap, dinvw_ap, src16_ap, dstf_ap, dinvs_ap = ins
    out_ap = outs[0]

    N, IN, SH, W, NT, SPLIT = (
        meta["N"],
        meta["IN"],
        meta["SH"],
        meta["W"],
        meta["NT"],
        meta["split"],
    )
    n_cores = meta["n_cores"]
    chunks = meta["chunks"]
    f32 = mybir.dt.float32
    AT = mybir.ActivationFunctionType
    OP = mybir.AluOpType

    MAX_CT = max(ch["tiles"] for ch in chunks)

    import contextlib

    with contextlib.ExitStack() as ctx:
        const = ctx.enter_context(tc.tile_pool(name="const", bufs=1))
        dram = ctx.enter_context(tc.tile_pool(name="dram", bufs=1, space="DRAM"))
        ohp = ctx.enter_context(tc.tile_pool(name="oh", bufs=12))
        psum_s = ctx.enter_context(tc.tile_pool(name="psum_s", bufs=3, space="PSUM"))
        psum_g = ctx.enter_context(tc.tile_pool(name="psum_g", bufs=2, space="PSUM"))
        tailp = ctx.enter_context(tc.tile_pool(name="tail", bufs=6))

        # persistent constants in SBUF
        iota_sb = const.tile([P, P], f32)
        nc.sync.dma_start(iota_sb[:], iota_ap[:])
        w1_sb = const.tile([IN, HID], f32)
        nc.sync.dma_start(w1_sb[:], w1_ap[:])
        w2_sb = const.tile([HID, OUT], f32)
        nc.sync.dma_start(w2_sb[:], w2_ap[:])
        b1_sb = const.tile([P, HID], f32)
        nc.sync.dma_start(b1_sb[:], b1_ap[:])
        b2_sb = const.tile([P, OUT], f32)
        nc.sync.dma_start(b2_sb[:], b2_ap[:])
        dinvw_sb = const.tile([P, W], f32)
        nc.sync.dma_start(dinvw_sb[:], dinvw_ap[:])
        src16_sb = const.tile([P, NT * 8], mybir.dt.int16)
        nc.sync.dma_start(src16_sb[:], src16_ap[:])
        dstf_sb = const.tile([P, NT], f32)
        nc.sync.dma_start(dstf_sb[:], dstf_ap[:])
        dinvs_sb = const.tile([P, NT], f32)
        nc.sync.dma_start(dinvs_sb[:], dinvs_ap[:])

        h1p_shard = dram.tile([SH, HID], f32)
        h1p_full = dram.tile([N, HID], f32)

        import os as _os
        _VARIANT = _os.environ.get("GCN_VARIANT", "full")
        NSWQ = int(_os.environ.get("GCN_NSWQ", "4"))
        TOKBUFS = int(_os.environ.get("GCN_TOKBUFS", "4"))
        qctr = [0]
        tokp_shared = ctx.enter_context(tc.tile_pool(name="tokp", bufs=TOKBUFS))

        def layer(F, src_dram, is_l1):
            """One GCN layer: gathers F-wide rows, segment-sums, returns via
            writer callback per window."""
            tokp = tokp_shared
            for ch in chunks:
                ct = ch["tiles"]
                t0 = ch["t0"]
                tokt = tokp.tile(
                    [P, MAX_CT, F], f32, tag="tok",
                    name=f"tok_{1 if is_l1 else 2}_{ch['t0']}",
                )
                # gather calls: lo then hi sections of this chunk
                nlo, nhi = ch["lo_tiles"], ch["hi_tiles"]
                GMAX = 8  # <=1024 idxs per call keeps single_packet mode legal
                for a0, n_all, src_ap in (
                    (0, nlo, src_dram[0:SPLIT, :]),
                    (nlo, nhi, src_dram[SPLIT:, :]),
                ):
                    for a in range(a0, a0 + n_all, GMAX):
                        b = min(a + GMAX, a0 + n_all)
                        nc.gpsimd.dma_gather(
                            out_ap=tokt[:, a:b, :],
                            in_ap=src_ap,
                            idxs_ap=src16_sb[:, (t0 + a) * 8 : (t0 + b) * 8],
                            num_idxs=(b - a) * P,
                            num_idxs_reg=(b - a) * P,
                            elem_size=F,
                            single_packet=True,
                            queue_num=qctr[0] % NSWQ,
                        )
                        qctr[0] += 1
                # per-window segment-sum matmuls
                win_psum = {}
                for w in ch["ws"]:
                  for ti in ch["wtiles"][w]:
                    tm = ch["tmeta"][ti]
                    gt = t0 + ti
                    d0, span = tm["d0"], tm["span"]
                    if tm["first"]:
                        win_psum[w] = psum_s.tile(
                            [P if is_l1 else HID, P], f32, tag="S", name=f"S_{w}"
                        )
                    oh = ohp.tile([P, P], f32, tag="oh")
                    if is_l1:
                        nc.vector.tensor_scalar(
                            out=oh[:, :span],
                            in0=iota_sb[:, d0 : d0 + span],
                            scalar1=dstf_sb[:, gt : gt + 1],
                            scalar2=dinvs_sb[:, gt : gt + 1],
                            op0=OP.is_equal,
                            op1=OP.mult,
                        )
                    else:
                        nc.vector.tensor_scalar(
                            out=oh[:, :span],
                            in0=iota_sb[:, d0 : d0 + span],
                            scalar1=dstf_sb[:, gt : gt + 1],
                            scalar2=None,
                            op0=OP.is_equal,
                        )
                    nc.tensor.matmul(
                        out=win_psum[w][:, d0 : d0 + span],
                        lhsT=tokt[:, ti, :],
                        rhs=oh[:, :span],
                        start=tm["first"],
                        stop=tm["last"],
                        skip_group_check=True,
                    )
                    if tm["last"]:
                        tail(w, win_psum[w], F, is_l1)
                    if _VARIANT == "gatheronly":
                        break

        def tail(w, s_psum, F, is_l1):
            dw = min(P, SH - w * P)
            # S.T -> SBUF
            p1t = tailp.tile([F, P], f32, tag=f"pt{1 if is_l1 else 2}")
            nc.vector.tensor_copy(out=p1t[:], in_=s_psum[:])
            wsb = w1_sb if is_l1 else w2_sb
            HO = HID if is_l1 else OUT
            g = psum_g.tile([P, HO], f32, tag="G")
            nc.tensor.matmul(
                out=g[:dw, :],
                lhsT=p1t[:, :dw],
                rhs=wsb[:],
                start=True,
                stop=True,
            )
            t1 = tailp.tile([P, HO], f32, tag=f"t1_{1 if is_l1 else 2}")
            # t1 = dinv_d * (S @ W)
            nc.vector.tensor_scalar(
                out=t1[:dw, :],
                in0=g[:dw, :],
                scalar1=dinvw_sb[:dw, w : w + 1],
                scalar2=None,
                op0=OP.mult,
            )
            t2 = tailp.tile([P, HO], f32, tag=f"t2_{1 if is_l1 else 2}")
            bsb = b1_sb if is_l1 else b2_sb
            nc.vector.tensor_tensor(
                out=t2[:dw, :], in0=t1[:dw, :], in1=bsb[:dw, :], op=OP.add
            )
            if is_l1:
                # h1p = dinv * relu(t2) == relu(dinv * t2)
                h1p_t = tailp.tile([P, HID], f32, tag="h1p")
                nc.scalar.activation(
                    out=h1p_t[:dw, :],
                    in_=t2[:dw, :],
                    func=AT.Relu,
                    scale=dinvw_sb[:dw, w : w + 1],
                )
                nc.sync.dma_start(
                    h1p_shard[w * P : w * P + dw, :], h1p_t[:dw, :]
                )
            else:
                nc.sync.dma_start(out_ap[w * P : w * P + dw, :], t2[:dw, :])

        layer(IN, x_ap, True)
        if _VARIANT in ("l1only", "gatheronly"):
            return
        if _os.environ.get("GCN_NOCOLL", "0") == "1":
            for _c in range(n_cores):
                nc.gpsimd.dma_start(
                    h1p_full[_c * SH : (_c + 1) * SH, :], h1p_shard[:]
                )
        else:
            nc.gpsimd.collective_compute(
                "AllGather",
                mybir.AluOpType.bypass,
                replica_groups=[list(range(n_cores))],
                ins=[h1p_shard[:]],
                outs=[h1p_full[:]],
            )
        layer(HID, h1p_full[:], False)


def make_inputs(x, W1, b1, W2, b2, meta, per_core):
    """Build the per-core input pytrees (ordered list)."""
    IN, W, NT = meta["IN"], meta["W"], meta["NT"]
    HID = W1.shape[1]
    OUT = W2.shape[1]
    iota = np.tile(np.arange(P, dtype=np.float32)[None, :], (P, 1))
    b1bc = np.tile(np.asarray(b1, np.float32)[None, :], (P, 1))
    b2bc = np.tile(np.asarray(b2, np.float32)[None, :], (P, 1))
    ins_list = []
    for pc in per_core:
        ins_list.append(
            [
                np.asarray(x, np.float32),
                np.asarray(W1, np.float32),
                np.asarray(W2, np.float32),
                b1bc,
                b2bc,
                iota,
                pc["dinvw"],
                pc["src16"],
                pc["dstf"],
                pc["dinvs"],
            ]
        )
    return ins_list


IN_NAMES = [
    "x", "w1", "w2", "b1bc", "b2bc", "iota", "dinvw", "src16", "dstf", "dinvs",
]


def run(x, W1, b1, W2, b2, edge_index, n_cores=8, split=32768, trace=False):
    import concourse.tile as tile
    from concourse import bacc, bass_utils, mybir
    from concourse.bass_interp import get_hw_module

    meta, per_core = preprocess(np.asarray(x), np.asarray(edge_index), n_cores, split)
    HID = W1.shape[1]
    OUT = W2.shape[1]
    SH = meta["SH"]
    ins_list = make_inputs(x, W1, b1, W2, b2, meta, per_core)

    import os as _os
    nc = bacc.Bacc(
        "TRN2", target_bir_lowering=False, debug=False, num_devices=n_cores,
        num_swdge_queues=int(_os.environ.get("GCN_NSWQ", "4")),
    )
    in_aps = [
        nc.dram_tensor(nm, list(a.shape), mybir.dt.from_np(a.dtype),
                       kind="ExternalInput").ap()
        for nm, a in zip(IN_NAMES, ins_list[0])
    ]
    out_t = nc.dram_tensor("out", [SH, OUT], mybir.dt.float32,
                           kind="ExternalOutput")
    with tile.TileContext(nc) as tc:
        build_kernel(tc, [out_t.ap()], in_aps, meta, HID, OUT)
    nc.compile()

    in_maps = [
        {nm: np.ascontiguousarray(a) for nm, a in zip(IN_NAMES, arrs)}
        for arrs in ins_list
    ]
    old_m = nc.m
    nc.m = get_hw_module(nc.m)
    try:
        res = bass_utils.run_bass_kernel_spmd(
            nc, in_maps, core_ids=list(range(n_cores)), trace=False
        )
        bench_ns = _bench(nc, in_maps, n_cores) if trace else None
    finally:
        nc.m = old_m
    out = np.concatenate([res.results[c]["out"] for c in range(n_cores)], axis=0)
    return out, res, bench_ns


def _bench(nc, in_maps, n_cores, iters=30):
    """Repeat-execute the compiled NEFF with device-resident inputs and
    return the min wall-clock ns per execution (upper bound on HW time)."""
    import time

    import jax
    from concourse import bass2jax, mybir
    from jax.sharding import Mesh, PartitionSpec
    from jax.experimental.shard_map import shard_map

    part_name = nc.partition_id_tensor.name if nc.partition_id_tensor else None
    in_names, out_names, out_avals, zero_outs = [], [], [], []
    for alloc in nc.m.functions[0].allocations:
        if not isinstance(alloc, bass2jax.mybir.MemoryLocationSet):
            continue
        name = alloc.memorylocations[0].name
        if alloc.kind == "ExternalInput":
            if name != part_name:
                in_names.append(name)
        elif alloc.kind == "ExternalOutput":
            out_names.append(name)
            shape = tuple(alloc.tensor_shape)
            dtype = bass2jax.mybir.dt.np(alloc.dtype)
            out_avals.append(jax.core.ShapedArray(shape, dtype))
            zero_outs.append(np.zeros(shape, dtype))
    n_params = len(in_names)
    all_names = in_names + out_names
    if part_name is not None:
        all_names = all_names + [part_name]

    def _make_body(nchain):
        def _body(*args):
            ins = list(args[:n_params])
            outs = list(args[n_params:])
            for _ in range(nchain):
                operands = ins + outs
                if part_name is not None:
                    operands.append(bass2jax.partition_id_tensor())
                outs = list(
                    bass2jax._bass_exec_p.bind(
                        *operands,
                        out_avals=tuple(out_avals),
                        in_names=tuple(all_names),
                        out_names=tuple(out_names),
                        lowering_input_output_aliases=(),
                        sim_require_finite=True,
                        sim_require_nnan=True,
                        nc=nc,
                    )
                )
            return tuple(outs)

        return _body

    devices = jax.devices()[:n_cores]
    mesh = Mesh(np.asarray(devices), ("core",))
    nio = n_params + len(out_names)
    sh = jax.sharding.NamedSharding(mesh, PartitionSpec("core"))
    concat_in = [
        jax.device_put(
            np.concatenate([in_maps[c][nm] for c in range(n_cores)], axis=0), sh
        )
        for nm in in_names
    ]
    concat_zero = [
        jax.device_put(np.zeros((n_cores * z.shape[0], *z.shape[1:]), z.dtype), sh)
        for z in zero_outs
    ]

    def time_chain(nchain, reps):
        fn = jax.jit(
            shard_map(
                _make_body(nchain),
                mesh=mesh,
                in_specs=(PartitionSpec("core"),) * nio,
                out_specs=(PartitionSpec("core"),) * len(out_names),
                check_rep=False,
            ),
            keep_unused=True,
        )
        r = fn(*concat_in, *concat_zero)
        jax.block_until_ready(r)
        best = float("inf")
        for _ in range(reps):
            t0 = time.perf_counter()
            r = fn(*concat_in, *concat_zero)
            jax.block_until_ready(r)
            best = min(best, time.perf_counter() - t0)
        return best

    fn = jax.jit(
        shard_map(
            _make_body(1),
            mesh=mesh,
            in_specs=(PartitionSpec("core"),) * nio,
            out_specs=(PartitionSpec("core"),) * len(out_names),
            check_rep=False,
        ),
        keep_unused=True,
    )
    base_fn = jax.jit(lambda a: a[0:1, 0:1] * 2.0)
    jax.block_until_ready(fn(*concat_in, *concat_zero))
    jax.block_until_ready(base_fn(concat_in[0]))
    # interleave kernel/baseline so terminal-load drift cancels in the delta
    deltas = []
    for _ in range(iters):
        t0 = time.perf_counter()
        jax.block_until_ready(base_fn(concat_in[0]))
        t1 = time.perf_counter()
        jax.block_until_ready(fn(*concat_in, *concat_zero))
        t2 = time.perf_counter()
        jax.block_until_ready(base_fn(concat_in[0]))
        t3 = time.perf_counter()
        # kernel minus mean of bracketing baselines
        deltas.append((t2 - t1) - ((t1 - t0) + (t3 - t2)) / 2.0)
    deltas.sort()
    med = deltas[len(deltas) // 2]
    per_exec = max(0.0, med)
    print(f"[bench] interleaved median delta={med*1e6:.1f}us "
          f"min={deltas[0]*1e6:.1f}us max={deltas[-1]*1e6:.1f}us")
    return int(per_exec * 1e9)


def kernel(x, W1, b1, W2, b2, edge_index):
    out, _, _ = run(
        np.asarray(x, np.float32),
        np.asarray(W1, np.float32),
        np.asarray(b1, np.float32),
        np.asarray(W2, np.float32),
        np.asarray(b2, np.float32),
        np.asarray(edge_index, np.int32),
    )
    return out


# revision 24
# speedup vs baseline: 1.3205x; 1.1651x over previous
"""2-layer GCN (GCNConv x2, relu between) on 8 Trainium2 NeuronCores.

Strategy (graph-parallel, per the sharding hint):
  - Nodes are partitioned into 8 contiguous shards by dst; each core owns the
    edges incident (by dst) to its shard, plus one self-loop token per node.
  - Aggregation is transform-last:  out = dinv_d * ((sum_e dinv_src*x[src]) @ W) + b
    so the per-edge work is a pure row gather (dma_gather) + a segment-sum
    done on the TensorEngine via narrow one-hot matrices built on the DVE.
  - Self-loops are ordinary edges (src == dst).
  - Layer 1 gathers raw x rows and folds dinv[src] into the one-hot values.
    Layer 2 gathers h1p = dinv * relu(layer1) rows (pre-scaled), one-hot is 0/1.
  - h1p shards are AllGather'd across the 8 cores between the layers.

The Bass program is SPMD: one program, per-core data. All per-core arrays are
padded to common shapes; token slots are padded with (idx=0, dstf=-1, dinvs=0)
so pads gather real rows but contribute exactly zero.
"""

import sys

if "/opt/trn_rl_repo" not in sys.path:
    sys.path.insert(0, "/opt/trn_rl_repo")

import numpy as np

P = 128


def _cdiv(a, b):
    return (a + b - 1) // b


def preprocess(x, edge_index, n_cores, split):
    """Host-side graph preprocessing. Returns (meta, per_core_arrays)."""
    N, IN = x.shape
    src = edge_index[0].astype(np.int64)
    dst = edge_index[1].astype(np.int64)
    SH = N // n_cores
    assert SH * n_cores == N
    W = _cdiv(SH, P)

    deg = (np.bincount(dst, minlength=N).astype(np.float32) + 1.0).astype(np.float32)
    dinv = (1.0 / np.sqrt(deg)).astype(np.float32)

    # per (core, window, grp) token lists, sorted by local dst
    # grp 0: src < split (gather source x[:split]); grp 1: src >= split
    tok = {}  # (c, w, g) -> (src_arr, dloc_arr)
    cnt = np.zeros((n_cores, W, 2), np.int64)
    for c in range(n_cores):
        m = (dst >= c * SH) & (dst < (c + 1) * SH)
        s_c = np.concatenate([src[m], np.arange(c * SH, (c + 1) * SH)])
        d_c = np.concatenate([dst[m] - c * SH, np.arange(SH)])
        w_c = d_c >> 7
        dl_c = d_c & 127
        g_c = (s_c >= split).astype(np.int64)
        order = np.lexsort((dl_c, g_c, w_c))
        s_c, dl_c, w_c, g_c = s_c[order], dl_c[order], w_c[order], g_c[order]
        # boundaries
        key = (w_c * 2 + g_c)
        for kv in range(2 * W):
            sel = key == kv
            w, g = kv // 2, kv % 2
            tok[(c, w, g)] = (s_c[sel], dl_c[sel])
            cnt[c, w, g] = sel.sum()

    LO_T = [int(max(_cdiv(int(cnt[c, w, 0]), P) for c in range(n_cores))) for w in range(W)]
    HI_T = [int(max(_cdiv(int(cnt[c, w, 1]), P) for c in range(n_cores))) for w in range(W)]

    # chunking: CHUNK_W windows per chunk; stream order per chunk:
    #   [w0.lo tiles, w1.lo, ...][w0.hi, w1.hi, ...]
    CHUNK_W = 4
    chunks = []  # list of dicts describing each chunk
    NT = 0  # total tiles
    tile_meta = []  # per global tile: dict(w, first, last)
    for c0 in range(0, W, CHUNK_W):
        ws = list(range(c0, min(c0 + CHUNK_W, W)))
        lo_tiles = sum(LO_T[w] for w in ws)
        hi_tiles = sum(HI_T[w] for w in ws)
        ch = {
            "ws": ws,
            "t0": NT,
            "lo_tiles": lo_tiles,
            "hi_tiles": hi_tiles,
            "tiles": lo_tiles + hi_tiles,
        }
        # per-tile window assignment
        tmeta = []
        for g in (0, 1):
            for w in ws:
                ntl = (LO_T, HI_T)[g][w]
                for i in range(ntl):
                    tmeta.append({"w": w, "g": g, "i": i})
        # mark first/last tile of each window (within this chunk; windows don't
        # span chunks)
        seen_first = set()
        last_idx = {}
        for ti, tm in enumerate(tmeta):
            w = tm["w"]
            tm["first"] = w not in seen_first
            seen_first.add(w)
            last_idx[w] = ti
        for ti, tm in enumerate(tmeta):
            tm["last"] = last_idx[tm["w"]] == ti
        ch["tmeta"] = tmeta
        # per-window iteration order (lo tiles then hi tiles of that window)
        ch["wtiles"] = {
            w: [ti for ti, tm in enumerate(tmeta) if tm["w"] == w] for w in ws
        }
        chunks.append(ch)
        tile_meta.extend(tmeta)
        NT += len(tmeta)

    NTOK = NT * P

    # build per-core flat slot arrays + per-tile span (union over cores)
    d0_arr = np.full(NT, 127, np.int64)
    d1_arr = np.zeros(NT, np.int64)
    per_core = []
    for c in range(n_cores):
        idx_local = np.zeros(NTOK, np.int32)
        dstf = np.full(NTOK, -1.0, np.float32)
        dinvs = np.zeros(NTOK, np.float32)
        for ch in chunks:
            base = ch["t0"]
            for ti, tm in enumerate(ch["tmeta"]):
                gt = base + ti
                w, g, i = tm["w"], tm["g"], tm["i"]
                s_all, dl_all = tok[(c, w, g)]
                a, b = i * P, min((i + 1) * P, len(s_all))
                n = max(0, b - a)
                o = gt * P
                if n > 0:
                    sv = s_all[a:b]
                    dv = dl_all[a:b]
                    idx_local[o : o + n] = sv - (split if g else 0)
                    dstf[o : o + n] = dv.astype(np.float32)
                    dinvs[o : o + n] = dinv[sv]
                    d0_arr[gt] = min(d0_arr[gt], int(dv.min()))
                    d1_arr[gt] = max(d1_arr[gt], int(dv.max()))
                if n < P:
                    # pads: idx 0 is valid in both lo and hi views
                    idx_local[o + n : o + P] = 0
        # int16 wrap layout [128, NTOK//16], token j -> (j%16, j//16), replicated x8
        assert idx_local.max() < 32768 and idx_local.min() >= 0
        i16 = idx_local.astype(np.int16).reshape(NTOK // 16, 16).T  # [16, NTOK//16]
        i16 = np.tile(i16, (8, 1)).copy()  # [128, NTOK//16]
        dstf_sb = dstf.reshape(NT, P).T.copy()  # [128, NT]
        dinvs_sb = dinvs.reshape(NT, P).T.copy()
        # per-window per-node dinv [128, W]
        dinvw = np.ones((P, W), np.float32)
        for w in range(W):
            dw = min(P, SH - w * P)
            dinvw[:dw, w] = dinv[c * SH + w * P : c * SH + w * P + dw]
        per_core.append(
            {"src16": i16, "dstf": dstf_sb, "dinvs": dinvs_sb, "dinvw": dinvw}
        )

    # tile spans (compile-time, shared across cores)
    spans = []
    for gt in range(NT):
        if tile_meta[gt]["first"]:
            spans.append((0, P))
        else:
            d0 = int(d0_arr[gt]) & ~1
            d1 = int(d1_arr[gt])
            if d1 < d0:  # all-pad tile on every core
                d0, d1 = 0, 1
            end = min(P, (d1 + 2) & ~1)
            spans.append((d0, end - d0))
    for gt in range(NT):
        tile_meta[gt]["d0"], tile_meta[gt]["span"] = spans[gt]

    meta = {
        "N": N,
        "IN": IN,
        "SH": SH,
        "W": W,
        "NT": NT,
        "split": split,
        "n_cores": n_cores,
        "chunks": chunks,
        "LO_T": LO_T,
        "HI_T": HI_T,
    }
    return meta, per_core


def build_kernel(tc, outs, ins, meta, HID, OUT):
    """Trace the SPMD program. ins order:
    x, w1, w2, b1bc, b2bc, iota, dinvw, src16, dstf, dinvs"""
    import concourse.bass as bass
    from concourse import mybir

    nc = tc.nc
    x_ap, w1_ap, w2_ap, b1_ap, b2_ap, iota_ap, dinvw_ap, src16_ap, dstf_ap, dinvs_ap = ins
    out_ap = outs[0]

    N, IN, SH, W, NT, SPLIT = (
        meta["N"],
        meta["IN"],
        meta["SH"],
        meta["W"],
        meta["NT"],
        meta["split"],
    )
    n_cores = meta["n_cores"]
    chunks = meta["chunks"]
    f32 = mybir.dt.float32
    AT = mybir.ActivationFunctionType
    OP = mybir.AluOpType

    MAX_CT = max(ch["tiles"] for ch in chunks)

    import contextlib

    with contextlib.ExitStack() as ctx:
        const = ctx.enter_context(tc.tile_pool(name="const", bufs=1))
        dram = ctx.enter_context(tc.tile_pool(name="dram", bufs=1, space="DRAM"))
        ohp = ctx.enter_context(tc.tile_pool(name="oh", bufs=12))
        psum_s = ctx.enter_context(tc.tile_pool(name="psum_s", bufs=3, space="PSUM"))
        psum_g = ctx.enter_context(tc.tile_pool(name="psum_g", bufs=2, space="PSUM"))
        tailp = ctx.enter_context(tc.tile_pool(name="tail", bufs=6))

        # persistent constants in SBUF
        iota_sb = const.tile([P, P], f32)
        nc.sync.dma_start(iota_sb[:], iota_ap[:])
        w1_sb = const.tile([IN, HID], f32)
        nc.sync.dma_start(w1_sb[:], w1_ap[:])
        w2_sb = const.tile([HID, OUT], f32)
        nc.sync.dma_start(w2_sb[:], w2_ap[:])
        b1_sb = const.tile([P, HID], f32)
        nc.sync.dma_start(b1_sb[:], b1_ap[:])
        b2_sb = const.tile([P, OUT], f32)
        nc.sync.dma_start(b2_sb[:], b2_ap[:])
        dinvw_sb = const.tile([P, W], f32)
        nc.sync.dma_start(dinvw_sb[:], dinvw_ap[:])
        src16_sb = const.tile([P, NT * 8], mybir.dt.int16)
        nc.sync.dma_start(src16_sb[:], src16_ap[:])
        dstf_sb = const.tile([P, NT], f32)
        nc.sync.dma_start(dstf_sb[:], dstf_ap[:])
        dinvs_sb = const.tile([P, NT], f32)
        nc.sync.dma_start(dinvs_sb[:], dinvs_ap[:])

        h1p_shard = dram.tile([SH, HID], f32)
        h1p_full = dram.tile([N, HID], f32)

        import os as _os
        _VARIANT = _os.environ.get("GCN_VARIANT", "full")
        NSWQ = int(_os.environ.get("GCN_NSWQ", "4"))
        TOKBUFS = int(_os.environ.get("GCN_TOKBUFS", "4"))
        qctr = [0]
        tokp_shared = ctx.enter_context(tc.tile_pool(name="tokp", bufs=TOKBUFS))

        def layer(F, src_dram, is_l1):
            """One GCN layer: gathers F-wide rows, segment-sums, returns via
            writer callback per window."""
            tokp = tokp_shared
            for ch in chunks:
                ct = ch["tiles"]
                t0 = ch["t0"]
                tokt = tokp.tile(
                    [P, MAX_CT, F], f32, tag="tok",
                    name=f"tok_{1 if is_l1 else 2}_{ch['t0']}",
                )
                # gather calls: lo then hi sections of this chunk
                nlo, nhi = ch["lo_tiles"], ch["hi_tiles"]
                GMAX = 8  # <=1024 idxs per call keeps single_packet mode legal
                for a0, n_all, src_ap in (
                    (0, nlo, src_dram[0:SPLIT, :]),
                    (nlo, nhi, src_dram[SPLIT:, :]),
                ):
                    for a in range(a0, a0 + n_all, GMAX):
                        b = min(a + GMAX, a0 + n_all)
                        nc.gpsimd.dma_gather(
                            out_ap=tokt[:, a:b, :],
                            in_ap=src_ap,
                            idxs_ap=src16_sb[:, (t0 + a) * 8 : (t0 + b) * 8],
                            num_idxs=(b - a) * P,
                            num_idxs_reg=(b - a) * P,
                            elem_size=F,
                            single_packet=True,
                            queue_num=qctr[0] % NSWQ,
                        )
                        qctr[0] += 1
                # per-window segment-sum matmuls
                win_psum = {}
                for w in ch["ws"]:
                  for ti in ch["wtiles"][w]:
                    tm = ch["tmeta"][ti]
                    gt = t0 + ti
                    d0, span = tm["d0"], tm["span"]
                    if tm["first"]:
                        win_psum[w] = psum_s.tile(
                            [P if is_l1 else HID, P], f32, tag="S", name=f"S_{w}"
                        )
                    oh = ohp.tile([P, P], f32, tag="oh")
                    if is_l1:
                        nc.vector.tensor_scalar(
                            out=oh[:, :span],
                            in0=iota_sb[:, d0 : d0 + span],
                            scalar1=dstf_sb[:, gt : gt + 1],
                            scalar2=dinvs_sb[:, gt : gt + 1],
                            op0=OP.is_equal,
                            op1=OP.mult,
                        )
                    else:
                        nc.vector.tensor_scalar(
                            out=oh[:, :span],
                            in0=iota_sb[:, d0 : d0 + span],
                            scalar1=dstf_sb[:, gt : gt + 1],
                            scalar2=None,
                            op0=OP.is_equal,
                        )
                    nc.tensor.matmul(
                        out=win_psum[w][:, d0 : d0 + span],
                        lhsT=tokt[:, ti, :],
                        rhs=oh[:, :span],
                        start=tm["first"],
                        stop=tm["last"],
                        skip_group_check=True,
                    )
                    if tm["last"]:
                        tail(w, win_psum[w], F, is_l1)
                    if _VARIANT == "gatheronly":
                        break

        def tail(w, s_psum, F, is_l1):
            dw = min(P, SH - w * P)
            # S.T -> SBUF
            p1t = tailp.tile([F, P], f32, tag=f"pt{1 if is_l1 else 2}")
            nc.vector.tensor_copy(out=p1t[:], in_=s_psum[:])
            wsb = w1_sb if is_l1 else w2_sb
            HO = HID if is_l1 else OUT
            g = psum_g.tile([P, HO], f32, tag="G")
            nc.tensor.matmul(
                out=g[:dw, :],
                lhsT=p1t[:, :dw],
                rhs=wsb[:],
                start=True,
                stop=True,
            )
            t1 = tailp.tile([P, HO], f32, tag=f"t1_{1 if is_l1 else 2}")
            # t1 = dinv_d * (S @ W)
            nc.vector.tensor_scalar(
                out=t1[:dw, :],
                in0=g[:dw, :],
                scalar1=dinvw_sb[:dw, w : w + 1],
                scalar2=None,
                op0=OP.mult,
            )
            t2 = tailp.tile([P, HO], f32, tag=f"t2_{1 if is_l1 else 2}")
            bsb = b1_sb if is_l1 else b2_sb
            nc.vector.tensor_tensor(
                out=t2[:dw, :], in0=t1[:dw, :], in1=bsb[:dw, :], op=OP.add
            )
            if is_l1:
                # h1p = dinv * relu(t2) == relu(dinv * t2)
                h1p_t = tailp.tile([P, HID], f32, tag="h1p")
                nc.scalar.activation(
                    out=h1p_t[:dw, :],
                    in_=t2[:dw, :],
                    func=AT.Relu,
                    scale=dinvw_sb[:dw, w : w + 1],
                )
                nc.sync.dma_start(
                    h1p_shard[w * P : w * P + dw, :], h1p_t[:dw, :]
                )
            else:
                nc.sync.dma_start(out_ap[w * P : w * P + dw, :], t2[:dw, :])

        layer(IN, x_ap, True)
        if _VARIANT in ("l1only", "gatheronly"):
            return
        if _os.environ.get("GCN_NOCOLL", "0") == "1":
            for _c in range(n_cores):
                nc.gpsimd.dma_start(
                    h1p_full[_c * SH : (_c + 1) * SH, :], h1p_shard[:]
                )
        else:
            nc.gpsimd.collective_compute(
                "AllGather",
                mybir.AluOpType.bypass,
                replica_groups=[list(range(n_cores))],
                ins=[h1p_shard[:]],
                outs=[h1p_full[:]],
            )
        layer(HID, h1p_full[:], False)


def make_inputs(x, W1, b1, W2, b2, meta, per_core):
    """Build the per-core input pytrees (ordered list)."""
    IN, W, NT = meta["IN"], meta["W"], meta["NT"]
    HID = W1.shape[1]
    OUT = W2.shape[1]
    iota = np.tile(np.arange(P, dtype=np.float32)[None, :], (P, 1))
    b1bc = np.tile(np.asarray(b1, np.float32)[None, :], (P, 1))
    b2bc = np.tile(np.asarray(b2, np.float32)[None, :], (P, 1))
    ins_list = []
    for pc in per_core:
        ins_list.append(
            [
                np.asarray(x, np.float32),
                np.asarray(W1, np.float32),
                np.asarray(W2, np.float32),
                b1bc,
                b2bc,
                iota,
                pc["dinvw"],
                pc["src16"],
                pc["dstf"],
                pc["dinvs"],
            ]
        )
    return ins_list


IN_NAMES = [
    "x", "w1", "w2", "b1bc", "b2bc", "iota", "dinvw", "src16", "dstf", "dinvs",
]


def run(x, W1, b1, W2, b2, edge_index, n_cores=8, split=32768, trace=False):
    import concourse.tile as tile
    from concourse import bacc, bass_utils, mybir
    from concourse.bass_interp import get_hw_module

    meta, per_core = preprocess(np.asarray(x), np.asarray(edge_index), n_cores, split)
    HID = W1.shape[1]
    OUT = W2.shape[1]
    SH = meta["SH"]
    ins_list = make_inputs(x, W1, b1, W2, b2, meta, per_core)

    import os as _os
    nc = bacc.Bacc(
        "TRN2", target_bir_lowering=False, debug=False, num_devices=n_cores,
        num_swdge_queues=int(_os.environ.get("GCN_NSWQ", "4")),
    )
    in_aps = [
        nc.dram_tensor(nm, list(a.shape), mybir.dt.from_np(a.dtype),
                       kind="ExternalInput").ap()
        for nm, a in zip(IN_NAMES, ins_list[0])
    ]
    out_t = nc.dram_tensor("out", [SH, OUT], mybir.dt.float32,
                           kind="ExternalOutput")
    with tile.TileContext(nc) as tc:
        build_kernel(tc, [out_t.ap()], in_aps, meta, HID, OUT)
    nc.compile()

    in_maps = [
        {nm: np.ascontiguousarray(a) for nm, a in zip(IN_NAMES, arrs)}
        for arrs in ins_list
    ]
    old_m = nc.m
    nc.m = get_hw_module(nc.m)
    try:
        res = bass_utils.run_bass_kernel_spmd(
            nc, in_maps, core_ids=list(range(n_cores)), trace=False
        )
        bench_ns = _bench(nc, in_maps, n_cores) if trace else None
    finally:
        nc.m = old_m
    out = np.concatenate([res.results[c]["out"] for c in range(n_cores)], axis=0)
    return out, res, bench_ns


def _bench(nc, in_maps, n_cores, iters=30):
    """Repeat-execute the compiled NEFF with device-resident inputs and
    return the min wall-clock ns per execution (upper bound on HW time)."""
    import time

    import jax
    from concourse import bass2jax, mybir
    from jax.sharding import Mesh, PartitionSpec
    from jax.experimental.shard_map import shard_map

    part_name = nc.partition_id_tensor.name if nc.partition_id_tensor else None
    in_names, out_names, out_avals, zero_outs = [], [], [], []
    for alloc in nc.m.functions[0].allocations:
        if not isinstance(alloc, bass2jax.mybir.MemoryLocationSet):
            continue
        name = alloc.memorylocations[0].name
        if alloc.kind == "ExternalInput":
            if name != part_name:
                in_names.append(name)
        elif alloc.kind == "ExternalOutput":
            out_names.append(name)
            shape = tuple(alloc.tensor_shape)
            dtype = bass2jax.mybir.dt.np(alloc.dtype)
            out_avals.append(jax.core.ShapedArray(shape, dtype))
            zero_outs.append(np.zeros(shape, dtype))
    n_params = len(in_names)
    all_names = in_names + out_names
    if part_name is not None:
        all_names = all_names + [part_name]

    def _make_body(nchain):
        def _body(*args):
            ins = list(args[:n_params])
            outs = list(args[n_params:])
            for _ in range(nchain):
                operands = ins + outs
                if part_name is not None:
                    operands.append(bass2jax.partition_id_tensor())
                outs = list(
                    bass2jax._bass_exec_p.bind(
                        *operands,
                        out_avals=tuple(out_avals),
                        in_names=tuple(all_names),
                        out_names=tuple(out_names),
                        lowering_input_output_aliases=(),
                        sim_require_finite=True,
                        sim_require_nnan=True,
                        nc=nc,
                    )
                )
            return tuple(outs)

        return _body

    devices = jax.devices()[:n_cores]
    mesh = Mesh(np.asarray(devices), ("core",))
    nio = n_params + len(out_names)
    sh = jax.sharding.NamedSharding(mesh, PartitionSpec("core"))
    concat_in = [
        jax.device_put(
            np.concatenate([in_maps[c][nm] for c in range(n_cores)], axis=0), sh
        )
        for nm in in_names
    ]
    concat_zero = [
        jax.device_put(np.zeros((n_cores * z.shape[0], *z.shape[1:]), z.dtype), sh)
        for z in zero_outs
    ]

    def time_chain(nchain, reps):
        fn = jax.jit(
            shard_map(
                _make_body(nchain),
                mesh=mesh,
                in_specs=(PartitionSpec("core"),) * nio,
                out_specs=(PartitionSpec("core"),) * len(out_names),
                check_rep=False,
            ),
            keep_unused=True,
        )
        r = fn(*concat_in, *concat_zero)
        jax.block_until_ready(r)
        best = float("inf")
        for _ in range(reps):
            t0 = time.perf_counter()
            r = fn(*concat_in, *concat_zero)
            jax.block_until_ready(r)
            best = min(best, time.perf_counter() - t0)
        return best

    fn = jax.jit(
        shard_map(
            _make_body(1),
            mesh=mesh,
            in_specs=(PartitionSpec("core"),) * nio,
            out_specs=(PartitionSpec("core"),) * len(out_names),
            check_rep=False,
        ),
        keep_unused=True,
    )
    base_fn = jax.jit(lambda a: a[0:1, 0:1] * 2.0)
    jax.block_until_ready(fn(*concat_in, *concat_zero))
    jax.block_until_ready(base_fn(concat_in[0]))
    # interleave kernel/baseline so terminal-load drift cancels in the delta
    deltas = []
    for _ in range(iters):
        t0 = time.perf_counter()
        jax.block_until_ready(base_fn(concat_in[0]))
        t1 = time.perf_counter()
        jax.block_until_ready(fn(*concat_in, *concat_zero))
        t2 = time.perf_counter()
        jax.block_until_ready(base_fn(concat_in[0]))
        t3 = time.perf_counter()
        # kernel minus mean of bracketing baselines
        deltas.append((t2 - t1) - ((t1 - t0) + (t3 - t2)) / 2.0)
    deltas.sort()
    med = deltas[len(deltas) // 2]
    per_exec = max(0.0, med)
    print(f"[bench] interleaved delta min={deltas[0]*1e6:.1f}us "
          f"median={med*1e6:.1f}us max={deltas[-1]*1e6:.1f}us")
    return int(per_exec * 1e9)


def kernel(x, W1, b1, W2, b2, edge_index):
    out, _, _ = run(
        np.asarray(x, np.float32),
        np.asarray(W1, np.float32),
        np.asarray(b1, np.float32),
        np.asarray(W2, np.float32),
        np.asarray(b2, np.float32),
        np.asarray(edge_index, np.int32),
    )
    return out
